# revision 29
# baseline (speedup 1.0000x reference)
"""AttentionBlock kernel for 8x Trainium2 NeuronCores — fp16 + fp8 DoubleRow.

Strategy: data-parallel over batch (B=8 -> 1 batch element per core).
Channel-major layout throughout (no on-chip transposes):

  q/k projection  [och, tok]  : fp16 (1 cycle/row, ~fp32 precision here)
  v  projection  [tok, och]   : fp16
  S^T = k^T q    [j, i]       : fp16 (K=128: fp8 DoubleRow gives no speedup)
  E = exp(scale*S^T - 3.5)    : ScalarE, fp8e4 out (max logit*scale ~8.5,
                                the -3.5 keeps E < 240 = fp8e4 max; the
                                softmax normalization cancels it exactly)
  O^T = sum_j v E             : fp8 DoubleRow (K=1024 -> 2x MAC rate)
  colsums (banded ones lhsT)  : fp8 DoubleRow, accumulated per head-pair
  normalize: DVE reciprocal (overlapped) + K=4 fp16 broadcast matmul + MUL
  out proj       [c, tok]     : fp8 DoubleRow (K=512)
  + bres + x residual (fp32) in one DVE op

The schedule weaves the S^T matmul stream just-in-time against the ScalarE
exp stream (the phase-C pacer), filling PE slack with the remaining
projection / attention-output / output-projection work.  S psum tiles come
from a dedicated 2-buffer pool so the weave, not psum recycling, sets the
PE lead over ScalarE.

Host-side prep: fp16/fp8 casts, DR pair-packing of Wo, bres = bo + Wo^T bv
(v bias folded through the output projection since sum_j softmax = 1).
"""

import sys

sys.path.insert(0, "/opt/trn_rl_repo")

import numpy as np

import concourse.bass as bass
import concourse.tile as tile
import concourse.mybir as mybir
from concourse.bass_utils import run_bass_kernel_spmd

B, C, HW = 8, 512, 1024
NH, DK = 4, 128
SCALE = float(DK) ** -0.5
EXPB = -3.5  # exp bias: E = exp(scale*s - 3.5); max logit*scale is ~8.5, keeps E < 240 (fp8e4 max)
F32 = mybir.dt.float32
F16 = mybir.dt.float16
BF16 = mybir.dt.bfloat16
FP8 = mybir.dt.float8e4
DR = mybir.MatmulPerfMode.DoubleRow

# ---------------------------------------------------------------------------
# Walrus in this container supports only ONE embedded sync-wait per
# instruction ("Too many sync wait commands" otherwise).  Tile emits
# multi-wait instructions, so rewrite: each instruction keeps its last wait
# and gets N-1 single-wait NoOps inserted right before it on the same engine.
# ---------------------------------------------------------------------------
_wsplit_counter = [0]


def _split_multi_waits(nc):
    for fn in nc.m.functions:
        for blk in fn.blocks:
            insts = blk.instructions
            if not insts:
                continue
            new = []
            changed = False
            for inst in insts:
                si = inst.sync_info
                waits = list(si.on_wait) if si is not None and si.on_wait else []
                if len(waits) > 1:
                    changed = True
                    for w in waits[:-1]:
                        _wsplit_counter[0] += 1
                        nop = mybir.InstNoOp(
                            name=f"WSPLIT-{_wsplit_counter[0]}",
                            ins=[],
                            outs=[],
                            engine=inst.engine,
                        )
                        nop.sync_info = mybir.SyncInfo(on_wait=[w], on_update=[])
                        nc.register_instruction(nop, overwrite=True)
                        new.append(nop)
                    inst.sync_info = mybir.SyncInfo(
                        on_wait=[waits[-1]], on_update=list(si.on_update or [])
                    )
                new.append(inst)
            if changed:
                blk.instructions = new


def build_attention_nc():
    nc = bass.Bass("TRN2")
    x16 = nc.dram_tensor("x16", [C, HW], F16, kind="ExternalInput")
    wqk = nc.dram_tensor("wqk", [C, 1024], F16, kind="ExternalInput")
    wv = nc.dram_tensor("wv", [C, 512], F16, kind="ExternalInput")
    wo8 = nc.dram_tensor("wo8", [128, 2 * 1024], FP8, kind="ExternalInput")
    bqk = nc.dram_tensor("bqk", [128, 9], F32, kind="ExternalInput")
    bres = nc.dram_tensor("bres", [128, 4], F32, kind="ExternalInput")
    tb8 = nc.dram_tensor("tb8", [128, 32], FP8, kind="ExternalInput")
    ub = nc.dram_tensor("ub", [4, 7 * 128], F16, kind="ExternalInput")
    out = nc.dram_tensor("out", [C, HW], F16, kind="ExternalOutput")

    x16, wqk, wv, wo8, bqk, bres, tb8, ub, out = (
        t.ap() for t in (x16, wqk, wv, wo8, bqk, bres, tb8, ub, out)
    )

    EXP = mybir.ActivationFunctionType.Exp
    ADD = mybir.AluOpType.add
    MUL = mybir.AluOpType.mult
    IC = [slice(0, 512), slice(512, 1024)]

    with tile.TileContext(nc) as tc:
        with (
            tc.tile_pool(name="persist", bufs=1) as persist,
            tc.tile_pool(name="epool", bufs=14) as epool,
            tc.tile_pool(name="outp", bufs=4) as outp,
            tc.tile_pool(name="psS", bufs=2, space="PSUM") as ps_s,
            tc.tile_pool(name="psM", bufs=2, space="PSUM") as ps_main,
            tc.tile_pool(name="psC", bufs=2, space="PSUM") as ps_cs,
        ):
            # ---- persistent SBUF tensors -------------------------------
            x_sb = [persist.tile([128, HW], F16, tag=f"x{i}", name=f"x{i}") for i in range(4)]
            wqk_sb = [persist.tile([128, 1024], F16, tag=f"wqk{i}", name=f"wqk{i}") for i in range(4)]
            wv_sb = [persist.tile([128, 512], F16, tag=f"wv{i}", name=f"wv{i}") for i in range(4)]
            wo_sb = [persist.tile([128, 2, 512], FP8, tag=f"wo{i}", name=f"wo{i}") for i in range(2)]
            # q^T / k^T per head: index h*2 + (0=q, 1=k)
            qk_sb = [persist.tile([128, HW], F16, tag=f"qk{i}", name=f"qk{i}") for i in range(8)]
            # v in [tok, 2 tok-tile, (h, d)] DR layout, 4 pair tiles
            v_sb = [persist.tile([128, 2, 512], FP8, tag=f"v{i}", name=f"v{i}") for i in range(4)]
            # attention output O^T (unnormalized bf16; normalized fp8 pairs)
            oT_sb = [persist.tile([128, HW], BF16, tag=f"oT{i}", name=f"oT{i}") for i in range(4)]
            o8_sb = [persist.tile([128, 2, 1024], FP8, tag=f"o8{i}", name=f"o8{i}") for i in range(2)]
            bqk_sb = persist.tile([128, 9], F32, tag="bqk", name="bqk_sb")
            bres_sb = persist.tile([128, 4], F32, tag="bres", name="bres_sb")
            tb_sb = persist.tile([128, 2, 16], FP8, tag="tb", name="tb_sb")
            u_sb = persist.tile([4, 7 * 128], F16, tag="u_sb", name="u_sb")
            csr = [persist.tile([4, 512], F16, tag=f"csr{i}", name=f"csr{i}") for i in range(2)]

            # ---- loads -------------------------------------------------
            # Startup is DMA-transfer-bound: spread x16 halves and wqk
            # column-chunks across all three DMA-capable queues so phase A
            # streams without p-state-dropping stalls.  The first A groups
            # need x[:, 0:512] halves + wqk cols 0:512, all landing ~9us.
            # critical set (x + wqk-lo, 1.5MB) balanced across all three
            # queues; wqk-hi (heads 2/3, needed later) and everything else
            # on gpsimd behind it.
            for half in range(2):
                for i in (0, 1):
                    nc.sync.dma_start(
                        out=x_sb[i][:, IC[half]],
                        in_=x16[i * 128 : (i + 1) * 128, IC[half]],
                    )
            nc.scalar.dma_start(out=bqk_sb, in_=bqk[:, :])
            for half in range(2):
                for i in (2, 3):
                    nc.scalar.dma_start(
                        out=x_sb[i][:, IC[half]],
                        in_=x16[i * 128 : (i + 1) * 128, IC[half]],
                    )
            for lohi in range(2):
                for kc in range(4):
                    nc.gpsimd.dma_start(
                        out=wqk_sb[kc][:, IC[lohi]],
                        in_=wqk[kc * 128 : (kc + 1) * 128, IC[lohi]],
                    )
            for kc in range(4):
                nc.sync.dma_start(
                    out=wv_sb[kc], in_=wv[kc * 128 : (kc + 1) * 128, :]
                )
            for p in range(2):
                nc.gpsimd.dma_start(
                    out=wo_sb[p],
                    in_=wo8[:, p * 1024 : (p + 1) * 1024].rearrange(
                        "p (two f) -> p two f", two=2
                    ),
                )
            nc.gpsimd.dma_start(out=bres_sb, in_=bres[:, :])
            nc.gpsimd.dma_start(
                out=tb_sb, in_=tb8[:, :].rearrange("p (two f) -> p two f", two=2)
            )
            nc.gpsimd.dma_start(out=u_sb, in_=ub[:, :])

            psc = [
                ps_cs.tile([4, 512], F32, tag="psC", name="psC")
                for _ in range(2)
            ]

            # trivial exp on a loaded const: forces the ACT table load (and
            # its ~1.3us cost) to happen during startup instead of right
            # before the first real exp.
            atl_scratch = persist.tile([1, 1], F32, tag="atl", name="atl")
            nc.scalar.activation(
                out=atl_scratch[:], in_=bqk_sb[0:1, 8:9], func=EXP
            )

            # ---- phase emitters ----------------------------------------
            def emit_Ag(h, qk, ic):
                """one q/k projection group: [128, 512] psum, 4 matmuls."""
                o0 = h * 256 + qk * 128
                hq = h * 2 + qk
                ps = ps_main.tile([128, 512], F32, tag="psM", name="psM")
                for kc in range(4):
                    nc.tensor.matmul(
                        ps[:],
                        wqk_sb[kc][:, o0 : o0 + 128],
                        x_sb[kc][:, IC[ic]],
                        start=(kc == 0),
                        stop=(kc == 3),
                    )
                nc.vector.tensor_scalar_add(
                    out=qk_sb[hq][:, IC[ic]],
                    in0=ps[:],
                    scalar1=bqk_sb[:, hq : hq + 1],
                )

            def emit_B(jp):
                """v projection for token-tile pair jp -> v_sb[jp]."""
                for jh in range(2):
                    jt = jp * 2 + jh
                    ps = ps_main.tile([128, 512], F32, tag="psM", name="psM")
                    for kc in range(4):
                        nc.tensor.matmul(
                            ps[:],
                            x_sb[kc][:, jt * 128 : (jt + 1) * 128],
                            wv_sb[kc][:],
                            start=(kc == 0),
                            stop=(kc == 3),
                        )
                    nc.vector.tensor_copy(out=v_sb[jp][:, jh, :], in_=ps[:])

            E_tiles = {h: [] for h in range(NH)}

            def emit_S(h, jt):
                """S^T tile (jt) for head h + its exp into the E pair tile."""
                qT = qk_sb[h * 2 + 0]
                kT = qk_sb[h * 2 + 1]
                jp, sl = jt // 2, jt % 2
                if sl == 0:
                    e = epool.tile([128, 2, 1024], FP8, tag="E", name="E")
                    E_tiles[h].append(e)
                e = E_tiles[h][jp]
                ps = ps_s.tile([128, HW], F32, tag="psS", name="psS")
                for ic in range(2):
                    nc.tensor.matmul(
                        ps[:, IC[ic]],
                        kT[:, jt * 128 : (jt + 1) * 128],
                        qT[:, IC[ic]],
                    )
                nc.scalar.activation(
                    out=e[:, sl, :], in_=ps[:], func=EXP,
                    scale=SCALE, bias=bqk_sb[:, 8:9],
                )

            pso = {}

            def emit_PV(h, jps=(0, 1, 2, 3)):
                """attn @ v for head h, fp8 DR over token pairs; ic-inner so
                consecutive matmuls share the stationary v tile."""
                if 0 in jps:
                    for ic in range(2):
                        pso[(h, ic)] = ps_main.tile([128, 512], F32, tag="psM", name="psM")
                E = E_tiles[h]
                for jp in jps:
                    for ic in range(2):
                        nc.tensor.matmul(
                            pso[(h, ic)][:],
                            v_sb[jp][:, :, h * 128 : (h + 1) * 128],
                            E[jp][:, :, ic * 512 : (ic + 1) * 512],
                            start=(jp == 0),
                            stop=(jp == 3),
                            perf_mode=DR,
                            skip_group_check=True,
                        )

            def emit_cs(h, ics=(0, 1)):
                """colsum accumulation for head h into psc[h//2], fp8 DR."""
                half = h // 2
                E = E_tiles[h]
                for ic in ics:
                    for jp in range(4):
                        q = (h % 2) * 2 + ic
                        nc.tensor.matmul(
                            psc[half][:],
                            tb_sb[:, :, q * 4 : q * 4 + 4],
                            E[jp][:, :, ic * 512 : (ic + 1) * 512],
                            start=(h % 2 == 0 and ic == 0 and jp == 0),
                            stop=(h % 2 == 1 and ic == 1 and jp == 3),
                            perf_mode=DR,
                            skip_group_check=True,
                        )

            def emit_oT_copy(h, eng=None):
                eng = eng or nc.vector
                for ic in range(2):
                    if eng is nc.scalar:
                        eng.activation(
                            out=oT_sb[h][:, IC[ic]], in_=pso[(h, ic)][:],
                            func=mybir.ActivationFunctionType.Copy,
                        )
                    else:
                        eng.tensor_copy(
                            out=oT_sb[h][:, IC[ic]], in_=pso[(h, ic)][:]
                        )

            def emit_recip(half):
                with nc.allow_low_precision(
                    reason="softmax denom reciprocal rounded to fp16"
                ):
                    nc.vector.reciprocal(out=csr[half][:], in_=psc[half][:])

            def emit_recip_act(half):
                """Reciprocal on ScalarE (its table also holds `copy`).  The
                DVE iterative reciprocal is ~6 cycles/element and the [4, 512]
                layout puts 512 elements on each of only 4 lanes (3.3us); the
                ScalarE table version takes ~0.6us + one table load and runs
                while the DVE handles the normalization multiplies."""
                e = nc.scalar
                ins = [
                    e.lower_ap(psc[half][:]),
                    mybir.ImmediateValue(dtype=mybir.dt.float32, value=0.0),
                    mybir.ImmediateValue(dtype=mybir.dt.float32, value=1.0),
                    mybir.ImmediateValue(dtype=mybir.dt.float32, value=0.0),
                ]
                outs = [e.lower_ap(csr[half][:])]
                return e.add_instruction(
                    mybir.InstActivation(
                        name=nc.get_next_instruction_name(),
                        func=mybir.ActivationFunctionType.Reciprocal,
                        ins=ins,
                        outs=outs,
                    )
                )

            def emit_bc(half, pool=None):
                """broadcast r over partitions (fp16 K=4 matmul) + normalize."""
                pool = pool or ps_main
                tag = "psS" if pool is ps_s else "psM"
                for ic in range(2):
                    for hh in range(2):
                        h = half * 2 + hh
                        q = hh * 2 + ic
                        bc = pool.tile([128, 512], F32, tag=tag, name="bc")
                        nc.tensor.matmul(
                            bc[:],
                            u_sb[:, (3 - q) * 128 : (4 - q) * 128],
                            csr[half][:],
                        )
                        nc.vector.tensor_tensor(
                            out=o8_sb[half][:, hh, ic * 512 : (ic + 1) * 512],
                            in0=oT_sb[h][:, IC[ic]], in1=bc[:],
                            op=MUL,
                        )

            psD = {}

            def emit_D(kc, hp, start, stop):
                for ic in range(2):
                    if hp == 0 and start:
                        psD[(kc, ic)] = ps_main.tile(
                            [128, 512], F32, tag="psM", name="psM"
                        )
                    nc.tensor.matmul(
                        psD[(kc, ic)][:],
                        wo_sb[hp][:, :, kc * 128 : (kc + 1) * 128],
                        o8_sb[hp][:, :, ic * 512 : (ic + 1) * 512],
                        start=start,
                        stop=stop,
                        perf_mode=DR,
                        skip_group_check=True,
                    )

            def emit_D_finish(kc):
                """psum -> SBUF with the bres bias + x residual.  kc 0-1 in
                one DVE op; kc 2-3 as ScalarE identity+bias then a gpsimd
                SBUF-only add, so the two halves drain on disjoint engines."""
                for ic in range(2):
                    ot = outp.tile([128, 512], F16, tag="out", name="out")
                    if kc >= 2:
                        th = outp.tile([128, 512], F16, tag="th", name="th")
                        nc.scalar.activation(
                            out=th[:], in_=psD[(kc, ic)][:],
                            func=mybir.ActivationFunctionType.Identity,
                            bias=bres_sb[:, kc : kc + 1],
                        )
                        nc.gpsimd.tensor_tensor(
                            out=ot[:], in0=th[:], in1=x_sb[kc][:, IC[ic]], op=ADD
                        )
                    else:
                        nc.vector.scalar_tensor_tensor(
                            out=ot[:],
                            in0=psD[(kc, ic)][:],
                            scalar=bres_sb[:, kc : kc + 1],
                            in1=x_sb[kc][:, IC[ic]],
                            op0=ADD,
                            op1=ADD,
                        )
                    q = nc.scalar if kc >= 2 else nc.sync
                    q.dma_start(
                        out=out[kc * 128 : (kc + 1) * 128, IC[ic]], in_=ot[:]
                    )

            # ---- schedule ----------------------------------------------
            # PE warmup: the tensor engine p-state ramps with sustained
            # use; tiny matmuls on a never-written scratch tile (garbage
            # values, result discarded) have no dependencies at all, so they
            # ramp the clock while the input DMAs stream.
            warm_sb = persist.tile([128, 8], F16, tag="warm", name="warm_sb")
            nc.vector.memset(warm_sb[:], 0.0)
            for w in range(56):
                wps = ps_main.tile([1, 8], F32, tag="psM", name="warm")
                nc.tensor.matmul(wps[:], warm_sb[:, 0:1], warm_sb[:, 0:8])

            # Weave the S/exp stream (ScalarE is the phase-C pacer) against
            # projection and attention-output work.  S(0, 0..3) only needs
            # q complete and the first half of k, so they start right after
            # three A groups.
            emit_Ag(0, 0, 0); emit_Ag(0, 0, 1); emit_Ag(0, 1, 0)
            emit_S(0, 0); emit_S(0, 1)
            emit_Ag(0, 1, 1)
            emit_S(0, 2); emit_S(0, 3)
            emit_Ag(1, 0, 0)
            emit_S(0, 4); emit_S(0, 5)
            emit_Ag(1, 0, 1)
            emit_S(0, 6); emit_S(0, 7)
            emit_Ag(1, 1, 0); emit_Ag(1, 1, 1)
            emit_S(1, 0); emit_S(1, 1)
            emit_Ag(2, 0, 0); emit_Ag(2, 0, 1)
            emit_S(1, 2); emit_S(1, 3)
            emit_Ag(2, 1, 0); emit_Ag(2, 1, 1)
            emit_S(1, 4); emit_S(1, 5)
            emit_Ag(3, 0, 0); emit_Ag(3, 0, 1)
            emit_S(1, 6); emit_S(1, 7)
            emit_Ag(3, 1, 0); emit_Ag(3, 1, 1)

            emit_S(2, 0); emit_S(2, 1)
            emit_B(0)
            emit_S(2, 2); emit_S(2, 3)
            emit_B(1)
            emit_S(2, 4); emit_S(2, 5)
            emit_B(2)
            emit_S(2, 6); emit_S(2, 7)
            emit_B(3)
            emit_S(3, 0); emit_S(3, 1)
            emit_PV(0, (0, 1))
            emit_S(3, 2)
            emit_PV(0, (2, 3))
            emit_S(3, 3)
            emit_cs(0, (0,))
            emit_oT_copy(0)
            emit_S(3, 4)
            emit_cs(0, (1,))
            emit_S(3, 5)
            emit_cs(1, (0,))
            emit_S(3, 6)
            emit_cs(1, (1,))
            emit_recip(0)
            emit_S(3, 7)
            emit_PV(1)
            emit_oT_copy(1)
            emit_bc(0)
            emit_cs(2)
            emit_PV(2)
            emit_oT_copy(2)
            emit_cs(3)
            emit_recip_act(1)  # ScalarE: table load already done post-exp
            emit_PV(3)
            emit_oT_copy(3, eng=nc.scalar)

            # D part 1 for kc0 + the above fill the recip/normalize window;
            # bc(1) draws psum from the (now idle) S pool so kc0's two
            # accumulating psD tiles can stay live in the main pool.
            emit_D(0, 0, start=True, stop=False)
            emit_bc(1, pool=ps_s)
            emit_D(0, 1, start=False, stop=True)
            emit_D_finish(0)
            for kc in (1, 2, 3):
                emit_D(kc, 0, start=True, stop=False)
                emit_D(kc, 1, start=False, stop=True)
                emit_D_finish(kc)

    _split_multi_waits(nc)
    return nc


_NC_CACHE = {}


def _get_nc():
    if "nc" not in _NC_CACHE:
        _NC_CACHE["nc"] = build_attention_nc()
    return _NC_CACHE["nc"]


def _pair_pack(a):
    """[512, F] channel-major -> [128, 2*2*F] DR pair-packed layout:
    out[part, (pair, ktile, f)] = a[pair*256 + ktile*128 + part, f]."""
    F = a.shape[1]
    return np.ascontiguousarray(
        a.reshape(2, 2, 128, F).transpose(2, 0, 1, 3).reshape(128, 4 * F)
    )


def _prep_inputs(x, Wp, bp, Wo, bo):
    f8 = mybir.dt.np(FP8)
    f16 = mybir.dt.np(F16)
    x = np.ascontiguousarray(x, dtype=np.float32)
    Wp = np.asarray(Wp, dtype=np.float32)
    bp = np.asarray(bp, dtype=np.float32).reshape(-1)
    Wo = np.asarray(Wo, dtype=np.float32)
    bo = np.asarray(bo, dtype=np.float32).reshape(-1)

    qk_idx = np.concatenate(
        [np.arange(h * 384, h * 384 + 256) for h in range(NH)]
    )
    v_idx = np.concatenate(
        [np.arange(h * 384 + 256, h * 384 + 384) for h in range(NH)]
    )
    wqk_f = np.ascontiguousarray(Wp[:, qk_idx]).astype(f16)  # [512, 1024]
    wv_f = np.ascontiguousarray(Wp[:, v_idx]).astype(f16)  # [512, 512]
    wo8 = _pair_pack(Wo).astype(f8)  # [128, 2048]  (pairs over d-rows)
    bqk_v = np.concatenate(
        [bp[qk_idx].reshape(8, 128).T, np.full((128, 1), EXPB)], axis=1
    )  # [128, 9]: col h*2+qk = bias, col 8 = exp bias const
    bqk_v = np.ascontiguousarray(bqk_v, dtype=np.float32)
    bv = bp[v_idx]
    bres = (bo + Wo.T @ bv).reshape(4, 128).T  # [128, 4], col kc
    bres = np.ascontiguousarray(bres, dtype=np.float32)

    # colsum band: per-q 4-col window (4-byte aligned) with ones at col q,
    # duplicated for both DR k-tiles
    tb = np.zeros((128, 32), dtype=f8)
    for q in range(4):
        for kt in range(2):
            tb[:, kt * 16 + q * 4 + q] = 1.0
    ubv = np.zeros((4, 7 * 128), dtype=f16)
    for k in range(4):
        ubv[k, (3 - k) * 128 : (4 - k) * 128] = 1.0
    return x, wqk_f, wv_f, wo8, bqk_v, bres, tb, ubv


def run_sharded(x, Wp, bp, Wo, bo, **spmd_kwargs):
    """Shard over batch, run on cores 0-7, gather.  Returns ([B,C,H,W], res)."""
    x, wqk_f, wv_f, wo8, bqk_v, bres, tb, ubv = _prep_inputs(x, Wp, bp, Wo, bo)
    f16 = mybir.dt.np(F16)

    nc = _get_nc()
    in_maps = []
    for b in range(B):
        xc = x[b].reshape(C, HW)
        in_maps.append(
            {
                "x16": xc.astype(f16),
                "wqk": wqk_f,
                "wv": wv_f,
                "wo8": wo8,
                "bqk": bqk_v,
                "bres": bres,
                "tb8": tb,
                "ub": ubv,
            }
        )
    res = run_bass_kernel_spmd(nc, in_maps, core_ids=list(range(B)), **spmd_kwargs)
    h = w = int(np.sqrt(HW))
    out = np.stack(
        [res.results[b]["out"].astype(np.float32).reshape(C, h, w) for b in range(B)]
    )
    return out, res


def kernel(x, Wp, bp, Wo, bo):
    out, _ = run_sharded(x, Wp, bp, Wo, bo)
    return out


# revision 30
# speedup vs baseline: 1.1357x; 1.1357x over previous
"""AttentionBlock kernel for 8x Trainium2 NeuronCores — fp16 + fp8 DoubleRow.

Strategy: data-parallel over batch (B=8 -> 1 batch element per core).
Channel-major layout throughout (no on-chip transposes):

  q/k projection  [och, tok]  : fp16 (1 cycle/row, ~fp32 precision here)
  v  projection  [tok, och]   : fp16
  S^T = k^T q    [j, i]       : fp16 (K=128: fp8 DoubleRow gives no speedup)
  E = exp(scale*S^T - 3.5)    : ScalarE, fp8e4 out (max logit*scale ~8.5,
                                the -3.5 keeps E < 240 = fp8e4 max; the
                                softmax normalization cancels it exactly)
  O^T = sum_j v E             : fp8 DoubleRow (K=1024 -> 2x MAC rate)
  colsums (banded ones lhsT)  : fp8 DoubleRow, accumulated per head-pair
  normalize: DVE reciprocal (overlapped) + K=4 fp16 broadcast matmul + MUL
  out proj       [c, tok]     : fp8 DoubleRow (K=512)
  + bres + x residual (fp32) in one DVE op

The schedule weaves the S^T matmul stream just-in-time against the ScalarE
exp stream (the phase-C pacer), filling PE slack with the remaining
projection / attention-output / output-projection work.  S psum tiles come
from a dedicated 2-buffer pool so the weave, not psum recycling, sets the
PE lead over ScalarE.

Host-side prep: fp16/fp8 casts, DR pair-packing of Wo, bres = bo + Wo^T bv
(v bias folded through the output projection since sum_j softmax = 1).
"""

import sys

sys.path.insert(0, "/opt/trn_rl_repo")

import numpy as np

import concourse.bass as bass
import concourse.tile as tile
import concourse.mybir as mybir
from concourse.bass_utils import run_bass_kernel_spmd

B, C, HW = 8, 512, 1024
NH, DK = 4, 128
SCALE = float(DK) ** -0.5
EXPB = -3.5  # exp bias: E = exp(scale*s - 3.5); max logit*scale is ~8.5, keeps E < 240 (fp8e4 max)
F32 = mybir.dt.float32
F16 = mybir.dt.float16
BF16 = mybir.dt.bfloat16
FP8 = mybir.dt.float8e4
DR = mybir.MatmulPerfMode.DoubleRow

# ---------------------------------------------------------------------------
# Walrus in this container supports only ONE embedded sync-wait per
# instruction ("Too many sync wait commands" otherwise).  Tile emits
# multi-wait instructions, so rewrite: each instruction keeps its last wait
# and gets N-1 single-wait NoOps inserted right before it on the same engine.
# ---------------------------------------------------------------------------
_wsplit_counter = [0]


def _split_multi_waits(nc):
    for fn in nc.m.functions:
        for blk in fn.blocks:
            insts = blk.instructions
            if not insts:
                continue
            new = []
            changed = False
            for inst in insts:
                si = inst.sync_info
                waits = list(si.on_wait) if si is not None and si.on_wait else []
                if len(waits) > 1:
                    changed = True
                    for w in waits[:-1]:
                        _wsplit_counter[0] += 1
                        nop = mybir.InstNoOp(
                            name=f"WSPLIT-{_wsplit_counter[0]}",
                            ins=[],
                            outs=[],
                            engine=inst.engine,
                        )
                        nop.sync_info = mybir.SyncInfo(on_wait=[w], on_update=[])
                        nc.register_instruction(nop, overwrite=True)
                        new.append(nop)
                    inst.sync_info = mybir.SyncInfo(
                        on_wait=[waits[-1]], on_update=list(si.on_update or [])
                    )
                new.append(inst)
            if changed:
                blk.instructions = new


def build_attention_nc():
    nc = bass.Bass("TRN2")
    x16 = nc.dram_tensor("x16", [C, HW], F16, kind="ExternalInput")
    wqk = nc.dram_tensor("wqk", [C, 1024], F16, kind="ExternalInput")
    wv = nc.dram_tensor("wv", [C, 512], F16, kind="ExternalInput")
    wo8 = nc.dram_tensor("wo8", [128, 2 * 1024], FP8, kind="ExternalInput")
    bqk = nc.dram_tensor("bqk", [128, 9], F32, kind="ExternalInput")
    bres = nc.dram_tensor("bres", [128, 4], F32, kind="ExternalInput")
    tb8 = nc.dram_tensor("tb8", [128, 32], FP8, kind="ExternalInput")
    ub = nc.dram_tensor("ub", [4, 7 * 128], F16, kind="ExternalInput")
    out = nc.dram_tensor("out", [C, HW], F16, kind="ExternalOutput")

    x16, wqk, wv, wo8, bqk, bres, tb8, ub, out = (
        t.ap() for t in (x16, wqk, wv, wo8, bqk, bres, tb8, ub, out)
    )

    EXP = mybir.ActivationFunctionType.Exp
    ADD = mybir.AluOpType.add
    MUL = mybir.AluOpType.mult
    IC = [slice(0, 512), slice(512, 1024)]

    with tile.TileContext(nc) as tc:
        with (
            tc.tile_pool(name="persist", bufs=1) as persist,
            tc.tile_pool(name="epool", bufs=14) as epool,
            tc.tile_pool(name="outp", bufs=4) as outp,
            tc.tile_pool(name="psS", bufs=2, space="PSUM") as ps_s,
            tc.tile_pool(name="psM", bufs=2, space="PSUM") as ps_main,
            tc.tile_pool(name="psC", bufs=2, space="PSUM") as ps_cs,
        ):
            # ---- persistent SBUF tensors -------------------------------
            x_sb = [persist.tile([128, HW], F16, tag=f"x{i}", name=f"x{i}") for i in range(4)]
            wqk_sb = [persist.tile([128, 1024], F16, tag=f"wqk{i}", name=f"wqk{i}") for i in range(4)]
            wv_sb = [persist.tile([128, 512], F16, tag=f"wv{i}", name=f"wv{i}") for i in range(4)]
            wo_sb = [persist.tile([128, 2, 512], FP8, tag=f"wo{i}", name=f"wo{i}") for i in range(2)]
            # q^T / k^T per head: index h*2 + (0=q, 1=k)
            qk_sb = [persist.tile([128, HW], F16, tag=f"qk{i}", name=f"qk{i}") for i in range(8)]
            # v in [tok, 2 tok-tile, (h, d)] DR layout, 4 pair tiles
            v_sb = [persist.tile([128, 2, 512], FP8, tag=f"v{i}", name=f"v{i}") for i in range(4)]
            # attention output O^T (unnormalized bf16; normalized fp8 pairs)
            oT_sb = [persist.tile([128, HW], BF16, tag=f"oT{i}", name=f"oT{i}") for i in range(4)]
            o8_sb = [persist.tile([128, 2, 1024], FP8, tag=f"o8{i}", name=f"o8{i}") for i in range(2)]
            bqk_sb = persist.tile([128, 9], F32, tag="bqk", name="bqk_sb")
            bres_sb = persist.tile([128, 4], F32, tag="bres", name="bres_sb")
            tb_sb = persist.tile([128, 2, 16], FP8, tag="tb", name="tb_sb")
            u_sb = persist.tile([4, 7 * 128], F16, tag="u_sb", name="u_sb")
            csr = [persist.tile([4, 512], F16, tag=f"csr{i}", name=f"csr{i}") for i in range(2)]

            # ---- loads -------------------------------------------------
            # Startup is DMA-transfer-bound: spread x16 halves and wqk
            # column-chunks across all three DMA-capable queues so phase A
            # streams without p-state-dropping stalls.  The first A groups
            # need x[:, 0:512] halves + wqk cols 0:512, all landing ~9us.
            # critical set (x + wqk-lo, 1.5MB) balanced across all three
            # queues; wqk-hi (heads 2/3, needed later) and everything else
            # on gpsimd behind it.
            for half in range(2):
                for i in (0, 1):
                    nc.sync.dma_start(
                        out=x_sb[i][:, IC[half]],
                        in_=x16[i * 128 : (i + 1) * 128, IC[half]],
                    )
            nc.scalar.dma_start(out=bqk_sb, in_=bqk[:, :])
            for half in range(2):
                for i in (2, 3):
                    nc.scalar.dma_start(
                        out=x_sb[i][:, IC[half]],
                        in_=x16[i * 128 : (i + 1) * 128, IC[half]],
                    )
            for lohi in range(2):
                for kc in range(4):
                    nc.gpsimd.dma_start(
                        out=wqk_sb[kc][:, IC[lohi]],
                        in_=wqk[kc * 128 : (kc + 1) * 128, IC[lohi]],
                    )
            for kc in range(4):
                nc.sync.dma_start(
                    out=wv_sb[kc], in_=wv[kc * 128 : (kc + 1) * 128, :]
                )
            for p in range(2):
                nc.gpsimd.dma_start(
                    out=wo_sb[p],
                    in_=wo8[:, p * 1024 : (p + 1) * 1024].rearrange(
                        "p (two f) -> p two f", two=2
                    ),
                )
            nc.gpsimd.dma_start(out=bres_sb, in_=bres[:, :])
            nc.gpsimd.dma_start(
                out=tb_sb, in_=tb8[:, :].rearrange("p (two f) -> p two f", two=2)
            )
            nc.gpsimd.dma_start(out=u_sb, in_=ub[:, :])

            psc = [
                ps_cs.tile([4, 512], F32, tag="psC", name="psC")
                for _ in range(2)
            ]

            # trivial exp on a loaded const: forces the ACT table load (and
            # its ~1.3us cost) to happen during startup instead of right
            # before the first real exp.
            atl_scratch = persist.tile([1, 1], F32, tag="atl", name="atl")
            nc.scalar.activation(
                out=atl_scratch[:], in_=bqk_sb[0:1, 8:9], func=EXP
            )

            # ---- phase emitters ----------------------------------------
            def emit_Ag(h, qk, ic):
                """one q/k projection group: [128, 512] psum, 4 matmuls."""
                o0 = h * 256 + qk * 128
                hq = h * 2 + qk
                ps = ps_main.tile([128, 512], F32, tag="psM", name="psM")
                for kc in range(4):
                    nc.tensor.matmul(
                        ps[:],
                        wqk_sb[kc][:, o0 : o0 + 128],
                        x_sb[kc][:, IC[ic]],
                        start=(kc == 0),
                        stop=(kc == 3),
                    )
                nc.vector.tensor_scalar_add(
                    out=qk_sb[hq][:, IC[ic]],
                    in0=ps[:],
                    scalar1=bqk_sb[:, hq : hq + 1],
                )

            def emit_B(jp):
                """v projection for token-tile pair jp -> v_sb[jp]."""
                for jh in range(2):
                    jt = jp * 2 + jh
                    ps = ps_main.tile([128, 512], F32, tag="psM", name="psM")
                    for kc in range(4):
                        nc.tensor.matmul(
                            ps[:],
                            x_sb[kc][:, jt * 128 : (jt + 1) * 128],
                            wv_sb[kc][:],
                            start=(kc == 0),
                            stop=(kc == 3),
                        )
                    nc.vector.tensor_copy(out=v_sb[jp][:, jh, :], in_=ps[:])

            E_tiles = {h: [] for h in range(NH)}

            def emit_S(h, jt):
                """S^T tile (jt) for head h + its exp into the E pair tile."""
                qT = qk_sb[h * 2 + 0]
                kT = qk_sb[h * 2 + 1]
                jp, sl = jt // 2, jt % 2
                if sl == 0:
                    e = epool.tile([128, 2, 1024], FP8, tag="E", name="E")
                    E_tiles[h].append(e)
                e = E_tiles[h][jp]
                ps = ps_s.tile([128, HW], F32, tag="psS", name="psS")
                for ic in range(2):
                    nc.tensor.matmul(
                        ps[:, IC[ic]],
                        kT[:, jt * 128 : (jt + 1) * 128],
                        qT[:, IC[ic]],
                    )
                nc.scalar.activation(
                    out=e[:, sl, :], in_=ps[:], func=EXP,
                    scale=SCALE, bias=bqk_sb[:, 8:9],
                )

            pso = {}

            def emit_PV(h, jps=(0, 1, 2, 3)):
                """attn @ v for head h, fp8 DR over token pairs; ic-inner so
                consecutive matmuls share the stationary v tile."""
                if 0 in jps:
                    for ic in range(2):
                        pso[(h, ic)] = ps_main.tile([128, 512], F32, tag="psM", name="psM")
                E = E_tiles[h]
                for jp in jps:
                    for ic in range(2):
                        nc.tensor.matmul(
                            pso[(h, ic)][:],
                            v_sb[jp][:, :, h * 128 : (h + 1) * 128],
                            E[jp][:, :, ic * 512 : (ic + 1) * 512],
                            start=(jp == 0),
                            stop=(jp == 3),
                            perf_mode=DR,
                            skip_group_check=True,
                        )

            def emit_cs(h, ics=(0, 1)):
                """colsum accumulation for head h into psc[h//2], fp8 DR."""
                half = h // 2
                E = E_tiles[h]
                for ic in ics:
                    for jp in range(4):
                        q = (h % 2) * 2 + ic
                        nc.tensor.matmul(
                            psc[half][:],
                            tb_sb[:, :, q * 4 : q * 4 + 4],
                            E[jp][:, :, ic * 512 : (ic + 1) * 512],
                            start=(h % 2 == 0 and ic == 0 and jp == 0),
                            stop=(h % 2 == 1 and ic == 1 and jp == 3),
                            perf_mode=DR,
                            skip_group_check=True,
                        )

            def emit_oT_copy(h, eng=None):
                eng = eng or nc.vector
                for ic in range(2):
                    if eng is nc.scalar:
                        eng.activation(
                            out=oT_sb[h][:, IC[ic]], in_=pso[(h, ic)][:],
                            func=mybir.ActivationFunctionType.Copy,
                        )
                    else:
                        eng.tensor_copy(
                            out=oT_sb[h][:, IC[ic]], in_=pso[(h, ic)][:]
                        )

            def emit_recip(half):
                with nc.allow_low_precision(
                    reason="softmax denom reciprocal rounded to fp16"
                ):
                    nc.vector.reciprocal(out=csr[half][:], in_=psc[half][:])

            def emit_recip_act(half):
                """Reciprocal on ScalarE (its table also holds `copy`).  The
                DVE iterative reciprocal is ~6 cycles/element and the [4, 512]
                layout puts 512 elements on each of only 4 lanes (3.3us); the
                ScalarE table version takes ~0.6us + one table load and runs
                while the DVE handles the normalization multiplies."""
                e = nc.scalar
                ins = [
                    e.lower_ap(psc[half][:]),
                    mybir.ImmediateValue(dtype=mybir.dt.float32, value=0.0),
                    mybir.ImmediateValue(dtype=mybir.dt.float32, value=1.0),
                    mybir.ImmediateValue(dtype=mybir.dt.float32, value=0.0),
                ]
                outs = [e.lower_ap(csr[half][:])]
                return e.add_instruction(
                    mybir.InstActivation(
                        name=nc.get_next_instruction_name(),
                        func=mybir.ActivationFunctionType.Reciprocal,
                        ins=ins,
                        outs=outs,
                    )
                )

            def emit_bc(half, pool=None):
                """broadcast r over partitions (fp16 K=4 matmul) + normalize."""
                pool = pool or ps_main
                tag = "psS" if pool is ps_s else "psM"
                for ic in range(2):
                    for hh in range(2):
                        h = half * 2 + hh
                        q = hh * 2 + ic
                        bc = pool.tile([128, 512], F32, tag=tag, name="bc")
                        nc.tensor.matmul(
                            bc[:],
                            u_sb[:, (3 - q) * 128 : (4 - q) * 128],
                            csr[half][:],
                        )
                        nc.vector.tensor_tensor(
                            out=o8_sb[half][:, hh, ic * 512 : (ic + 1) * 512],
                            in0=oT_sb[h][:, IC[ic]], in1=bc[:],
                            op=MUL,
                        )

            psD = {}

            def emit_D(kc, hp, start, stop):
                for ic in range(2):
                    if hp == 0 and start:
                        psD[(kc, ic)] = ps_main.tile(
                            [128, 512], F32, tag="psM", name="psM"
                        )
                    nc.tensor.matmul(
                        psD[(kc, ic)][:],
                        wo_sb[hp][:, :, kc * 128 : (kc + 1) * 128],
                        o8_sb[hp][:, :, ic * 512 : (ic + 1) * 512],
                        start=start,
                        stop=stop,
                        perf_mode=DR,
                        skip_group_check=True,
                    )

            def emit_D_finish(kc):
                """psum -> SBUF with the bres bias + x residual.  kc 0-1 in
                one DVE op; kc 2-3 as ScalarE identity+bias then a gpsimd
                SBUF-only add, so the two halves drain on disjoint engines."""
                for ic in range(2):
                    ot = outp.tile([128, 512], F16, tag="out", name="out")
                    if kc >= 2:
                        th = outp.tile([128, 512], F16, tag="th", name="th")
                        nc.scalar.activation(
                            out=th[:], in_=psD[(kc, ic)][:],
                            func=mybir.ActivationFunctionType.Identity,
                            bias=bres_sb[:, kc : kc + 1],
                        )
                        nc.gpsimd.tensor_tensor(
                            out=ot[:], in0=th[:], in1=x_sb[kc][:, IC[ic]], op=ADD
                        )
                    else:
                        nc.vector.scalar_tensor_tensor(
                            out=ot[:],
                            in0=psD[(kc, ic)][:],
                            scalar=bres_sb[:, kc : kc + 1],
                            in1=x_sb[kc][:, IC[ic]],
                            op0=ADD,
                            op1=ADD,
                        )
                    q = nc.scalar if kc >= 2 else nc.sync
                    q.dma_start(
                        out=out[kc * 128 : (kc + 1) * 128, IC[ic]], in_=ot[:]
                    )

            # ---- schedule ----------------------------------------------
            # PE warmup: the tensor engine p-state ramps with sustained
            # use; tiny matmuls on a never-written scratch tile (garbage
            # values, result discarded) have no dependencies at all, so they
            # ramp the clock while the input DMAs stream.
            warm_sb = persist.tile([128, 8], F16, tag="warm", name="warm_sb")
            nc.vector.memset(warm_sb[:], 0.0)

            def warm(n):
                for w in range(n):
                    wps = ps_main.tile([1, 8], F32, tag="psM", name="warm")
                    nc.tensor.matmul(wps[:], warm_sb[:, 0:1], warm_sb[:, 0:8])

            warm(40)

            # Weave the S/exp stream (ScalarE is the phase-C pacer) against
            # projection and attention-output work.  S(0, 0..3) only needs
            # q complete and the first half of k, so they start right after
            # three A groups.
            emit_Ag(0, 0, 0)
            warm(6)
            emit_Ag(0, 0, 1)
            warm(6)
            emit_Ag(0, 1, 0)
            emit_S(0, 0); emit_S(0, 1)
            emit_Ag(0, 1, 1)
            emit_S(0, 2); emit_S(0, 3)
            emit_Ag(1, 0, 0)
            emit_S(0, 4); emit_S(0, 5)
            emit_Ag(1, 0, 1)
            emit_S(0, 6); emit_S(0, 7)
            emit_Ag(1, 1, 0); emit_Ag(1, 1, 1)
            emit_S(1, 0); emit_S(1, 1)
            emit_Ag(2, 0, 0); emit_Ag(2, 0, 1)
            emit_S(1, 2); emit_S(1, 3)
            emit_Ag(2, 1, 0); emit_Ag(2, 1, 1)
            emit_S(1, 4); emit_S(1, 5)
            emit_Ag(3, 0, 0); emit_Ag(3, 0, 1)
            emit_S(1, 6); emit_S(1, 7)
            emit_Ag(3, 1, 0); emit_Ag(3, 1, 1)

            emit_S(2, 0); emit_S(2, 1)
            emit_B(0)
            emit_S(2, 2); emit_S(2, 3)
            emit_B(1)
            emit_S(2, 4); emit_S(2, 5)
            emit_B(2)
            emit_S(2, 6); emit_S(2, 7)
            emit_B(3)
            emit_S(3, 0); emit_S(3, 1)
            emit_PV(0, (0, 1))
            emit_S(3, 2)
            emit_PV(0, (2, 3))
            emit_S(3, 3)
            emit_cs(0, (0,))
            emit_oT_copy(0)
            emit_S(3, 4)
            emit_cs(0, (1,))
            emit_S(3, 5)
            emit_cs(1, (0,))
            emit_S(3, 6)
            emit_cs(1, (1,))
            emit_recip(0)
            emit_S(3, 7)
            emit_PV(1)
            emit_oT_copy(1)
            emit_bc(0)
            emit_cs(2)
            emit_PV(2)
            emit_oT_copy(2)
            emit_cs(3)
            emit_recip_act(1)  # ScalarE: table load already done post-exp
            emit_PV(3)
            emit_oT_copy(3, eng=nc.scalar)

            # D part 1 for kc0 + the above fill the recip/normalize window;
            # bc(1) draws psum from the (now idle) S pool so kc0's two
            # accumulating psD tiles can stay live in the main pool.
            emit_D(0, 0, start=True, stop=False)
            emit_bc(1, pool=ps_s)
            emit_D(0, 1, start=False, stop=True)
            emit_D_finish(0)
            for kc in (1, 2, 3):
                emit_D(kc, 0, start=True, stop=False)
                emit_D(kc, 1, start=False, stop=True)
                emit_D_finish(kc)

    _split_multi_waits(nc)
    return nc


_NC_CACHE = {}


def _get_nc():
    if "nc" not in _NC_CACHE:
        _NC_CACHE["nc"] = build_attention_nc()
    return _NC_CACHE["nc"]


def _pair_pack(a):
    """[512, F] channel-major -> [128, 2*2*F] DR pair-packed layout:
    out[part, (pair, ktile, f)] = a[pair*256 + ktile*128 + part, f]."""
    F = a.shape[1]
    return np.ascontiguousarray(
        a.reshape(2, 2, 128, F).transpose(2, 0, 1, 3).reshape(128, 4 * F)
    )


def _prep_inputs(x, Wp, bp, Wo, bo):
    f8 = mybir.dt.np(FP8)
    f16 = mybir.dt.np(F16)
    x = np.ascontiguousarray(x, dtype=np.float32)
    Wp = np.asarray(Wp, dtype=np.float32)
    bp = np.asarray(bp, dtype=np.float32).reshape(-1)
    Wo = np.asarray(Wo, dtype=np.float32)
    bo = np.asarray(bo, dtype=np.float32).reshape(-1)

    qk_idx = np.concatenate(
        [np.arange(h * 384, h * 384 + 256) for h in range(NH)]
    )
    v_idx = np.concatenate(
        [np.arange(h * 384 + 256, h * 384 + 384) for h in range(NH)]
    )
    wqk_f = np.ascontiguousarray(Wp[:, qk_idx]).astype(f16)  # [512, 1024]
    wv_f = np.ascontiguousarray(Wp[:, v_idx]).astype(f16)  # [512, 512]
    wo8 = _pair_pack(Wo).astype(f8)  # [128, 2048]  (pairs over d-rows)
    bqk_v = np.concatenate(
        [bp[qk_idx].reshape(8, 128).T, np.full((128, 1), EXPB)], axis=1
    )  # [128, 9]: col h*2+qk = bias, col 8 = exp bias const
    bqk_v = np.ascontiguousarray(bqk_v, dtype=np.float32)
    bv = bp[v_idx]
    bres = (bo + Wo.T @ bv).reshape(4, 128).T  # [128, 4], col kc
    bres = np.ascontiguousarray(bres, dtype=np.float32)

    # colsum band: per-q 4-col window (4-byte aligned) with ones at col q,
    # duplicated for both DR k-tiles
    tb = np.zeros((128, 32), dtype=f8)
    for q in range(4):
        for kt in range(2):
            tb[:, kt * 16 + q * 4 + q] = 1.0
    ubv = np.zeros((4, 7 * 128), dtype=f16)
    for k in range(4):
        ubv[k, (3 - k) * 128 : (4 - k) * 128] = 1.0
    return x, wqk_f, wv_f, wo8, bqk_v, bres, tb, ubv


def run_sharded(x, Wp, bp, Wo, bo, **spmd_kwargs):
    """Shard over batch, run on cores 0-7, gather.  Returns ([B,C,H,W], res)."""
    x, wqk_f, wv_f, wo8, bqk_v, bres, tb, ubv = _prep_inputs(x, Wp, bp, Wo, bo)
    f16 = mybir.dt.np(F16)

    nc = _get_nc()
    in_maps = []
    for b in range(B):
        xc = x[b].reshape(C, HW)
        in_maps.append(
            {
                "x16": xc.astype(f16),
                "wqk": wqk_f,
                "wv": wv_f,
                "wo8": wo8,
                "bqk": bqk_v,
                "bres": bres,
                "tb8": tb,
                "ub": ubv,
            }
        )
    res = run_bass_kernel_spmd(nc, in_maps, core_ids=list(range(B)), **spmd_kwargs)
    h = w = int(np.sqrt(HW))
    out = np.stack(
        [res.results[b]["out"].astype(np.float32).reshape(C, h, w) for b in range(B)]
    )
    return out, res


def kernel(x, Wp, bp, Wo, bo):
    out, _ = run_sharded(x, Wp, bp, Wo, bo)
    return out
